# revision 8
# baseline (speedup 1.0000x reference)
"""Trainium2 Bass kernel for EvolvingGraphLearner (GRU cell + pairwise graph conv).

Strategy: data-parallel over batch (B=16 -> 2 batches on each of 8 cores).
Per core, everything is computed feature-major (transposed) so matmuls never
need on-device transposes:
  - GRU cell: r/z via separate K=64 matmuls + sigmoid (keeps every elementwise
    op partition-0 aligned), K-split matmul + tanh, DVE elementwise.
  - Pairwise conv: for each sender group of 4 (and each conv), build the
    relu tile T[(s,h), j] = relu(psT[h,4g+s] + prT[h,j] + b[h]) with one fused
    tensor_scalar (add + max) on DVE/GPSIMD or ACT Relu (bias per partition),
    then reduce over h with a PE matmul using a block-diagonal-shifted fc1
    weight tile, accumulating 8 groups into one 32-partition PSUM strip.
  - Partition replication (R tiles) runs over SBUF->SBUF DMA; the per-group
    bias columns B[(s,h),(b,g)] = psT[h,4g+s] are built with identity matmuls
    into PSUM column strips (engines cannot move data across partitions).
  - Epilogue: (sup + b1) * sigmoid(msk + b2) from PSUM, DMA to DRAM.
"""

import numpy as np
import ml_dtypes

B, N, C, H = 16, 288, 32, 32
NCORES = 8
BPC = B // NCORES          # batches per core = 2
TOK = BPC * N              # 576 tokens per core
NG = N // 4                # 72 sender groups of 4 per batch

# engine schedule for relu-tile builds: cycled per group ("D"=DVE, "A"=ACT, "G"=GPSIMD)
T1_SCHED = "DDGDG"
T2_SCHED = "AGDAG"

# packed f32 weight layout (rows, cols in wpack [128, WPACK_COLS])
_WP = {
    "rW": (slice(0, 64), slice(0, 32)),
    "zW": (slice(0, 64), slice(32, 64)),
    "hcWa": (slice(0, 32), slice(64, 96)),
    "hcWb": (slice(0, 32), slice(96, 128)),
    "pW1s": (slice(0, 32), slice(128, 160)),
    "pW1r": (slice(0, 32), slice(160, 192)),
    "pW2s": (slice(0, 32), slice(192, 224)),
    "pW2r": (slice(0, 32), slice(224, 256)),
    "I32": (slice(0, 32), slice(256, 288)),
    "rb": (slice(0, 32), slice(288, 289)),
    "zb": (slice(0, 32), slice(289, 290)),
    "hcb": (slice(0, 32), slice(290, 291)),
    "cb1": (slice(0, 32), slice(291, 292)),
    "cb2": (slice(0, 32), slice(292, 293)),
    "bv1": (slice(0, 128), slice(293, 294)),
    "bv2": (slice(0, 128), slice(294, 295)),
}
WPACK_COLS = 295

_CACHE: dict = {}


def _build():
    import concourse.bacc as bacc
    import concourse.tile as tile
    from concourse import mybir

    F32 = mybir.dt.float32
    BF16 = mybir.dt.bfloat16
    AF = mybir.ActivationFunctionType
    OP = mybir.AluOpType

    nc = bacc.Bacc("TRN2", target_bir_lowering=False, debug=False,
                   num_devices=NCORES)

    xT = nc.dram_tensor("xT", [2 * H, TOK], F32, kind="ExternalInput").ap()
    wpack = nc.dram_tensor("wpack", [128, WPACK_COLS], F32, kind="ExternalInput").ap()
    wblk = nc.dram_tensor("wblk", [128, 512], BF16, kind="ExternalInput").ap()

    sup_d = nc.dram_tensor("sup", [TOK, N], F32, kind="ExternalOutput").ap()
    nsT_d = nc.dram_tensor("nsT", [H, TOK], F32, kind="ExternalOutput").ap()

    with tile.TileContext(nc) as tc:
        with tc.tile_pool(name="const", bufs=1) as cpool, \
             tc.tile_pool(name="work", bufs=3) as wpool, \
             tc.tile_pool(name="tb", bufs=6) as tbpool, \
             tc.tile_pool(name="ep", bufs=3) as eppool, \
             tc.tile_pool(name="psum", bufs=2, space="PSUM") as pspool:

            xTt = cpool.tile([2 * H, TOK], F32)
            nc.sync.dma_start(xTt[:], xT)
            stT = cpool.tile([H, TOK], F32)
            nc.sync.dma_start(stT[:], xT[C:C + H, :])
            wp = cpool.tile([128, WPACK_COLS], F32)
            nc.sync.dma_start(wp[:], wpack)
            wbt = cpool.tile([128, 512], BF16)
            nc.sync.dma_start(wbt[:], wblk)

            def W(name):
                r, c = _WP[name]
                return wp[r, c]

            R = {cv: cpool.tile([128, TOK], BF16, name=f"R{cv}") for cv in (1, 2)}
            Bt = {cv: cpool.tile([128, BPC, NG], F32, name=f"Bt{cv}") for cv in (1, 2)}
            psB = {cv: pspool.tile([128, BPC * NG], F32, tag="bb", bufs=2,
                                   name=f"psB{cv}") for cv in (1, 2)}
            nsTt = cpool.tile([H, TOK], F32)

            idx = {1: 0, 2: 0}

            def build(cv, Tt, b, bsl, g, eng):
                if eng == "D":
                    nc.vector.tensor_scalar(
                        Tt[:], R[cv][:, bsl], Bt[cv][:, b, g:g + 1], 0.0,
                        op0=OP.add, op1=OP.max)
                elif eng == "G":
                    nc.gpsimd.tensor_scalar(
                        Tt[:], R[cv][:, bsl], Bt[cv][:, b, g:g + 1], 0.0,
                        op0=OP.add, op1=OP.max)
                else:
                    nc.scalar.activation(
                        Tt[:], R[cv][:, bsl], AF.Relu,
                        bias=Bt[cv][:, b, g:g + 1])

            # ---- phase A: GRU cell + projections + R/B builds for both batches ----
            for b in range(BPC):
                bsl = slice(b * N, (b + 1) * N)

                ps_r = pspool.tile([H, N], F32, tag="aux", bufs=2, name="ps_r")
                nc.tensor.matmul(ps_r[:], W("rW"), xTt[:, bsl],
                                 start=True, stop=True, skip_group_check=True)
                ps_z = pspool.tile([H, N], F32, tag="aux", bufs=2, name="ps_z")
                nc.tensor.matmul(ps_z[:], W("zW"), xTt[:, bsl],
                                 start=True, stop=True, skip_group_check=True)
                r_s = wpool.tile([H, N], F32, tag="r_s")
                nc.scalar.activation(r_s[:], ps_r[:], AF.Sigmoid, bias=W("rb"))
                z_s = wpool.tile([H, N], F32, tag="z_s")
                nc.scalar.activation(z_s[:], ps_z[:], AF.Sigmoid, bias=W("zb"))
                rs = wpool.tile([H, N], F32, tag="rs")
                nc.vector.tensor_tensor(rs[:], r_s[:], stT[:, bsl], op=OP.mult)
                ps_hc = pspool.tile([H, N], F32, tag="aux", bufs=2, name="ps_hc")
                nc.tensor.matmul(ps_hc[:], W("hcWa"), xTt[0:C, bsl],
                                 start=True, stop=False, skip_group_check=True)
                nc.tensor.matmul(ps_hc[:], W("hcWb"), rs[:],
                                 start=False, stop=True, skip_group_check=True)
                hh = wpool.tile([H, N], F32, tag="hh")
                nc.scalar.activation(hh[:], ps_hc[:], AF.Tanh, bias=W("hcb"))
                dd = wpool.tile([H, N], F32, tag="dd")
                nc.vector.tensor_tensor(dd[:], stT[:, bsl], hh[:], op=OP.subtract)
                zd = wpool.tile([H, N], F32, tag="zd")
                nc.vector.tensor_tensor(zd[:], z_s[:], dd[:], op=OP.mult)
                nc.vector.tensor_tensor(nsTt[:, bsl], hh[:], zd[:], op=OP.add)
                aT = wpool.tile([H, N], F32, tag="aT")
                nc.vector.tensor_scalar_max(aT[:], nsTt[:, bsl], 0.0)
                nc.sync.dma_start(nsT_d[:, bsl], nsTt[:, bsl])

                for cv in (1, 2):
                    ps_ps = pspool.tile([H, N], F32, tag="aux", bufs=2, name="ps_ps")
                    nc.tensor.matmul(ps_ps[:], W(f"pW{cv}s"), aT[:],
                                     start=True, stop=True, skip_group_check=True)
                    ps_pr = pspool.tile([H, N], F32, tag="aux", bufs=2, name="ps_pr")
                    nc.tensor.matmul(ps_pr[:], W(f"pW{cv}r"), aT[:],
                                     start=True, stop=True, skip_group_check=True)
                    # receiver side: R rows 0:32 = prT + conv bias (bf16), then
                    # replicate to rows 32:128 with SBUF->SBUF DMA
                    nc.scalar.activation(R[cv][0:H, bsl], ps_pr[:], AF.Identity,
                                         bias=W(f"cb{cv}"))
                    nc.sync.dma_start(R[cv][H:2 * H, bsl], R[cv][0:H, bsl])
                    nc.sync.dma_start(R[cv][2 * H:4 * H, bsl], R[cv][0:2 * H, bsl])
                    # sender side: psT to SBUF, then B[(s,h),(b,g)] = psT[h,4g+s]
                    # via identity matmuls into PSUM column strips
                    psT = wpool.tile([H, N], F32, tag="psT")
                    nc.scalar.activation(psT[:], ps_ps[:], AF.Copy)
                    psT3 = psT.rearrange("h (g s) -> h g s", s=4)
                    for s in range(4):
                        nc.tensor.matmul(
                            psB[cv][32 * s:32 * (s + 1), b * NG:(b + 1) * NG],
                            W("I32"), psT3[:, :, s],
                            start=True, stop=True,
                            tile_position=(0, 32 * s),
                            skip_group_check=True)

            for cv in (1, 2):
                nc.vector.tensor_copy(Bt[cv][:, :, :], psB[cv].rearrange(
                    "p (b g) -> p b g", b=BPC))

            # ---- phase B: pairwise conv main loops ----
            for b in range(BPC):
                bsl = slice(b * N, (b + 1) * N)
                for t in range(3):
                    tp = 128 if t < 2 else 32
                    pssup = pspool.tile([tp, N], F32, tag="sup", name="pssup")
                    psmsk = pspool.tile([tp, N], F32, tag="msk", name="psmsk")
                    prev_mm = {1: None, 2: None}
                    for c_ in range(tp // 32):
                        for q in range(8):
                            g = t * 32 + c_ * 8 + q
                            T1 = tbpool.tile([128, N], BF16, tag="T1", name="T1")
                            build(1, T1, b, bsl, g, T1_SCHED[idx[1] % len(T1_SCHED)])
                            idx[1] += 1
                            m1 = nc.tensor.matmul(
                                pssup[32 * c_:32 * (c_ + 1), :],
                                wbt[:, 32 * q:32 * (q + 1)], T1[:],
                                start=(q == 0), stop=(q == 7),
                                tile_position=(0, 32 * c_),
                                skip_group_check=True)
                            # matmul start=True clears has_written for the whole
                            # bank: chunk groups sharing a bank must not reorder
                            if q == 0 and prev_mm[1] is not None:
                                tile.add_dep_helper(m1.ins, prev_mm[1].ins, sync=False,
                                                    reason="psum chunk order")
                            if q == 7:
                                prev_mm[1] = m1
                            T2 = tbpool.tile([128, N], BF16, tag="T2", name="T2")
                            build(2, T2, b, bsl, g, T2_SCHED[idx[2] % len(T2_SCHED)])
                            idx[2] += 1
                            m2 = nc.tensor.matmul(
                                psmsk[32 * c_:32 * (c_ + 1), :],
                                wbt[:, 256 + 32 * q:256 + 32 * (q + 1)], T2[:],
                                start=(q == 0), stop=(q == 7),
                                tile_position=(0, 32 * c_),
                                skip_group_check=True)
                            if q == 0 and prev_mm[2] is not None:
                                tile.add_dep_helper(m2.ins, prev_mm[2].ins, sync=False,
                                                    reason="psum chunk order")
                            if q == 7:
                                prev_mm[2] = m2
                    sg = eppool.tile([tp, N], F32, tag="sg", name="sg")
                    nc.scalar.activation(sg[:], psmsk[:], AF.Sigmoid, bias=W("bv2")[0:tp])
                    s1 = eppool.tile([tp, N], F32, tag="s1", name="s1")
                    nc.vector.tensor_scalar_add(s1[:], pssup[:], W("bv1")[0:tp])
                    ot = eppool.tile([tp, N], F32, tag="ot", name="ot")
                    nc.vector.tensor_tensor(ot[:], s1[:], sg[:], op=OP.mult)
                    row0 = b * N + t * 128
                    nc.sync.dma_start(sup_d[row0:row0 + tp, :], ot[:])

    nc.compile()
    return nc


def _get_nc():
    if "nc" not in _CACHE:
        _CACHE["nc"] = _build()
    return _CACHE["nc"]


def _prep_in_maps(inputs):
    f32 = np.float32
    bf16 = ml_dtypes.bfloat16
    inp = np.asarray(inputs["inputs"], f32)
    st = np.asarray(inputs["states"], f32)
    rz_W = np.asarray(inputs["rz_W"], f32)
    rz_b = np.asarray(inputs["rz_b"], f32)
    hc_W = np.asarray(inputs["hc_W"], f32)
    hc_b = np.asarray(inputs["hc_b"], f32)

    wp = np.zeros((128, WPACK_COLS), f32)

    def setw(name, arr):
        r, c = _WP[name]
        wp[r, c] = arr

    setw("rW", rz_W[:, :H])
    setw("zW", rz_W[:, H:])
    setw("rb", rz_b[:H].reshape(H, 1))
    setw("zb", rz_b[H:].reshape(H, 1))
    setw("hcWa", hc_W[:C])
    setw("hcWb", hc_W[C:])
    setw("hcb", hc_b.reshape(H, 1))
    setw("I32", np.eye(32, dtype=f32))
    wblk = np.zeros((128, 512), f32)
    for cv in (1, 2):
        W2 = np.asarray(inputs[f"c{cv}_fc2_W"], f32)
        b2 = np.asarray(inputs[f"c{cv}_fc2_b"], f32)
        W1 = np.asarray(inputs[f"c{cv}_fc1_W"], f32)
        b1 = np.asarray(inputs[f"c{cv}_fc1_b"], f32)
        setw(f"pW{cv}s", W2[:H])
        setw(f"pW{cv}r", W2[H:])
        setw(f"cb{cv}", b2.reshape(H, 1))
        setw(f"bv{cv}", np.full((128, 1), b1[0], f32))
        w1 = W1[:, 0]
        for q in range(8):
            for sl in range(4):
                m = 4 * q + sl
                wblk[32 * sl:32 * (sl + 1), (cv - 1) * 256 + 32 * q + m] = w1

    shared = {"wpack": wp, "wblk": wblk.astype(bf16)}
    in_maps = []
    for c in range(NCORES):
        bs = slice(c * BPC, (c + 1) * BPC)
        xT = np.empty((2 * H, TOK), f32)
        xT[:C] = inp[bs].transpose(2, 0, 1).reshape(C, TOK)
        xT[C:] = st[bs].transpose(2, 0, 1).reshape(H, TOK)
        in_maps.append({"xT": np.ascontiguousarray(xT), **shared})
    return in_maps


def kernel(**inputs):
    from concourse.bass_utils import run_bass_kernel_spmd

    nc = _get_nc()
    in_maps = _prep_in_maps(inputs)
    res = run_bass_kernel_spmd(nc, in_maps, core_ids=list(range(NCORES)))

    support = np.empty((B, N, N), np.float32)
    new_state = np.empty((B, N, H), np.float32)
    for c in range(NCORES):
        out = res.results[c]
        support[c * BPC:(c + 1) * BPC] = out["sup"].reshape(BPC, N, N)
        new_state[c * BPC:(c + 1) * BPC] = (
            out["nsT"].reshape(H, BPC, N).transpose(1, 2, 0))
    return support, new_state
